# revision 33
# baseline (speedup 1.0000x reference)
"""Trainium2 Bass kernel for triplane SO3 deformable attention.

Sharding: data-parallel over batch (8 batches -> 8 cores). Each core
processes 2048 queries against its own triplane.

v2 pipeline per core (per pair of 64-sample blocks = 128 samples):
  - dma_gather fp16 4-corner rows (1KB) for center + 8 rotated anchors
  - feat: stt@4x muls by ACT-broadcast weights, DVE in-place tree -> F
  - wsum = F @ W_wf via single matmul (lhsT = F^T)
  - mix: planes 0/1 stt@4x muls (ACT-broadcast alx) reduced by PE
    transpose-accumulate; plane 2 DVE 1x broadcast-mul + stt tree
  - out = M @ (W_v@W_o) + F via two accumulating matmuls, DMA out

Host side only shards, relayouts planes (fp16, 4-corner-dup rows),
computes gather indices / lerp weights, and folds projection weights.
"""

import os
import sys

import numpy as np

sys.path.insert(0, "/opt/trn_rl_repo")

import ml_dtypes  # noqa: E402, F401

import concourse.bacc as bacc  # noqa: E402
import concourse.bass as bass  # noqa: E402
import concourse.mybir as mybir  # noqa: E402
import concourse.tile as tile  # noqa: E402
from concourse import bass_utils  # noqa: E402
from concourse.library_config import mlp  # noqa: E402


def _install_ntff_hook():
    """Provide antenv.axon_hooks (absent in this image) so that
    run_bass_kernel_spmd(trace=True) can capture NTFF profiles via the
    axon PJRT .so. Mirrors trn_agent_boot/trn_boot.py step 6."""
    import types

    if "antenv.axon_hooks" in sys.modules:
        return True
    try:
        sys.path.insert(0, "/root/.axon_site/trn_agent_boot")
        import trn_boot  # noqa: E402

        hook = trn_boot._ntff_profile_via_ctypes("/opt/axon/libaxon_pjrt.so")
        if hook is None:
            return False
        mod = types.ModuleType("antenv.axon_hooks")
        mod._hook = hook
        mod.get_axon_ntff_profile_hook = lambda: mod._hook
        mod.set_axon_ntff_profile_hook = lambda h: setattr(mod, "_hook", h)
        sys.modules["antenv.axon_hooks"] = mod
        return True
    except Exception:
        return False

BS, NS, NCP, NH, C, HID, R = 8, 2048, 8, 8, 128, 128, 128
NBLK = NS // 64          # 32 blocks of 64 samples
NPAIR = NBLK // 2        # 16 pairs (128 samples each)
F16 = mybir.dt.float16
F32 = mybir.dt.float32
I16 = mybir.dt.int16
MULT = mybir.AluOpType.mult
ADD = mybir.AluOpType.add

_CACHE = {}


def _wrap_idx(flat):
    """int16 flat index list -> [128, N/16] wrapped+replicated dma_gather layout."""
    n = flat.shape[0]
    w = flat.reshape(n // 16, 16).T.astype(np.int16)  # [16, N/16], elem j at [j%16, j//16]
    return np.tile(w, (8, 1))


def _host_prep(inputs):
    q = np.asarray(inputs["query_pos"], dtype=np.float32)      # (8, 2048, 9)
    planes = [np.asarray(inputs[k], dtype=np.float32)
              for k in ("plane_xz", "plane_xy", "plane_yz")]    # (8, C, R, R)
    cp = np.asarray(inputs["control_points"], dtype=np.float32)  # (8, 3)
    W_v = np.asarray(inputs["W_v"], dtype=np.float32)
    W_w = np.asarray(inputs["W_w"], dtype=np.float32)
    W_o = np.asarray(inputs["W_o"], dtype=np.float32)

    # folded projections
    W_wf = W_w.reshape(C, NCP, NH).sum(axis=1)                  # (C, 8)
    W_vo = W_v @ W_o                                            # (C, C)

    # rotation 6d -> matrix (rows b1,b2,b3), all fp32
    a1, a2 = q[..., 3:6], q[..., 6:9]
    b1 = a1 / np.linalg.norm(a1, axis=-1, keepdims=True)
    b2 = a2 - np.sum(b1 * a2, axis=-1, keepdims=True) * b1
    b2 = b2 / np.linalg.norm(b2, axis=-1, keepdims=True)
    b3 = np.cross(b1, b2)
    rot = np.stack([b1, b2, b3], axis=-2)                       # (8, 2048, 3, 3)
    cpr = np.einsum("bnpd,gd->bngp", rot, cp).astype(np.float32)  # (8, 2048, 8, 3)
    pts = np.concatenate([q[:, :, None, :3], q[:, :, None, :3] + cpr], axis=2)
    # (8, 2048, 9, 3); anchor 0 = center

    coord_pairs = [(0, 2), (0, 1), (1, 2)]  # (x-dim, y-dim) for xz, xy, yz

    # static device constants
    pairc = np.eye(128, dtype=np.float16)  # fp16 identity

    xs = np.minimum(np.arange(R) + 1, R - 1)
    ys = np.minimum(np.arange(R) + 1, R - 1)

    core_inputs = []
    for b in range(BS):
        im = {"pairc": pairc,
              "wwf": W_wf.astype(np.float16), "wvo": W_vo.astype(np.float16)}
        idxf_all, idxm_all, w4_all = [], [], []
        for pi in range(3):
            P = planes[pi][b]                       # (C, R, R)
            PT = np.transpose(P, (1, 2, 0))         # (y, x, c)
            E = np.concatenate(
                [PT, PT[:, xs, :], PT[ys, :, :], PT[ys][:, xs, :]],
                axis=-1)                            # (R, R, 4C) 2x2 patches
            im[f"ep{pi}"] = np.ascontiguousarray(
                E.reshape(R * R, 4 * C)).astype(np.float16)

            cx, cy = coord_pairs[pi]
            u = pts[b, :, :, cx]                    # (2048, 9)
            v = pts[b, :, :, cy]
            x = np.clip(u, 0.0, 1.0).astype(np.float32) * np.float32(R - 1)
            y = np.clip(v, 0.0, 1.0).astype(np.float32) * np.float32(R - 1)
            x0 = np.floor(x); y0 = np.floor(y)
            fx = (x - x0).astype(np.float32); fy = (y - y0).astype(np.float32)
            x0i = x0.astype(np.int32); y0i = y0.astype(np.int32)
            idx = y0i * R + x0i                    # (2048, 9) patch row id

            # corner weights (2048, 9, 4) order (y0x0, y0x1, y1x0, y1x1)
            wy = np.stack([1.0 - fy, fy], axis=-1)
            wx = np.stack([1.0 - fx, fx], axis=-1)
            w4 = (wy[..., :, None] * wx[..., None, :]).reshape(NS, 9, 4)
            w4_all.append(w4.astype(np.float32))

            # feat indices: anchor 0, order (pair, s2) -> partition = s2
            af = idx[:, 0].reshape(NPAIR, 128).ravel()
            idxf_all.append(_wrap_idx(af))
            # mix indices: anchors 1..8, order (pair, a, s2)
            am = idx[:, 1:].reshape(NPAIR, 128, 8).transpose(0, 2, 1).ravel()
            idxm_all.append(_wrap_idx(am))

        # idxa: feat cols + first 4 pairs (pair-major), so the first
        # gathers only wait on a small index load; idxb: remaining pairs
        mixcol = lambda pr, p: idxm_all[p][:, pr * 64:(pr + 1) * 64]
        im["idxa"] = np.concatenate(
            idxf_all + [mixcol(pr, p) for pr in range(4) for p in range(3)],
            axis=1)                                  # [128, 1152]
        im["idxb"] = np.concatenate(
            [mixcol(pr, p) for pr in range(4, NPAIR) for p in range(3)],
            axis=1)                                  # [128, 2304]

        W4 = np.stack(w4_all, axis=2)                   # (2048, 9, 3, 4) [s,a,p,cor]
        gf = W4[:, 0].reshape(NPAIR, 128, 3, 4).transpose(1, 2, 0, 3)
        gf16 = np.ascontiguousarray(
            gf.reshape(128, NPAIR * 12)).astype(np.float16)  # (p, pair, cor)
        # duplicate each f16 weight into an f16-pair viewed as one f32 so the
        # on-device broadcast (per-element copy cost) moves half the elements
        im["gwf"] = np.ascontiguousarray(
            np.repeat(gf16[:, :, None], 2, axis=2)).view(np.float32)[:, :, 0]
        gm = W4[:, 1:].reshape(NPAIR, 128, 8, 3, 4).transpose(1, 0, 3, 2, 4)
        im["gwm"] = np.ascontiguousarray(
            gm.reshape(128, NPAIR * 96)).astype(np.float16)  # (pair, p, a, cor)
        core_inputs.append(im)
    return core_inputs


def _build():
    nc = bacc.Bacc("TRN2", target_bir_lowering=False, num_swdge_queues=4)
    ep = [nc.dram_tensor(f"ep{p}", [R * R, 4 * C], F16, kind="ExternalInput")
          for p in range(3)]
    idxa_d = nc.dram_tensor("idxa", [128, 1152], I16, kind="ExternalInput")
    idxb_d = nc.dram_tensor("idxb", [128, 2304], I16, kind="ExternalInput")
    gwf_d = nc.dram_tensor("gwf", [128, NPAIR * 12], F32, kind="ExternalInput")
    gwm_d = nc.dram_tensor("gwm", [128, NPAIR * 96], F16, kind="ExternalInput")
    pairc_d = nc.dram_tensor("pairc", [128, 128], F16, kind="ExternalInput")
    wwf_d = nc.dram_tensor("wwf", [C, NCP], F16, kind="ExternalInput")
    wvo_d = nc.dram_tensor("wvo", [C, C], F16, kind="ExternalInput")
    out_d = nc.dram_tensor("out", [NS, C], F32, kind="ExternalOutput")

    with tile.TileContext(nc) as tc:
        with (
            tc.tile_pool(name="const", bufs=1) as cpool,
            tc.tile_pool(name="gf", bufs=1) as gfpool,
            tc.tile_pool(name="gm", bufs=6) as gmpool,
            tc.tile_pool(name="al", bufs=NPAIR) as alpool,
            tc.tile_pool(name="ft", bufs=NPAIR) as ftpool,
            tc.tile_pool(name="wt", bufs=NPAIR) as wtpool,
            tc.tile_pool(name="mt", bufs=3) as mtpool,
            tc.tile_pool(name="osb", bufs=3) as opool,
            tc.tile_pool(name="psmisc", bufs=2, space="PSUM") as pmiscpool,
            tc.tile_pool(name="psmix", bufs=2, space="PSUM") as psmtpool,
            tc.tile_pool(name="pso", bufs=2, space="PSUM") as psopool,
        ):
            nc.gpsimd.load_library(mlp)

            def cload(name, dram, shape, dt, eng=None):
                t = cpool.tile(shape, dt, tag=name)
                (eng or nc.sync).dma_start(t[:], dram[:])
                return t

            # gather-critical index tensors load first; the small idxa
            # unblocks the first gathers early
            idxa_t = cload("idxa", idxa_d, [128, 1152], I16)
            idxb_t = cload("idxb", idxb_d, [128, 2304], I16)
            gwf_t = cload("gwf", gwf_d, [128, NPAIR * 12], F32)
            gwm_t = cload("gwm", gwm_d, [128, NPAIR * 96], F16)
            ident_t = cload("pairc", pairc_d, [128, 128], F16)
            wwf_t = cload("wwf", wwf_d, [C, NCP], F16)
            wvo_t = cload("wvo", wvo_d, [C, C], F16)

            # dma_gather crashes the exec unit above 1024 idx/call -> chunk
            qn = [0]

            def gather1k(dst, src_d, idx_t, col0, nidx):
                for h in range(nidx // 1024):
                    nc.gpsimd.dma_gather(
                        dst[:, h * 8:(h + 1) * 8, :], src_d[:],
                        idx_t[:, col0 + h * 64:col0 + (h + 1) * 64],
                        1024, 1024, 512, queue_num=qn[0] % 4)
                    qn[0] += 1

            # feat gathers: one patch row per sample, 8 pairs per call;
            # second half is issued after a few mix gathers so the ring
            # reuse of its tile never stalls the gather queue
            gfeat = {}

            def featgather(h):
                for p in range(3):
                    t = gfpool.tile([128, 8, 512], F16, tag=f"gfe{p}")
                    gather1k(t, ep[p], idxa_t, p * 128 + h * 64, 1024)
                    gfeat[(p, h)] = t

            # mix gathers: per (pair, plane): 8 anchors * 128 samples =
            # 1024 idx; all three planes land in one [128, 24, 512] tile
            gmix = {}

            def mixgather(ch):
                t = gmpool.tile([128, 24, 512], F16, tag="gmx")
                for p in range(3):
                    if ch < 4:
                        gather1k(t[:, p * 8:(p + 1) * 8, :], ep[p], idxa_t,
                                 384 + ch * 192 + p * 64, 1024)
                    else:
                        gather1k(t[:, p * 8:(p + 1) * 8, :], ep[p], idxb_t,
                                 (ch - 4) * 192 + p * 64, 1024)
                gmix[ch] = t

            featgather(0)
            for ch in range(4):
                mixgather(ch)
            featgather(1)
            for ch in range(4, NPAIR):
                mixgather(ch)

            npair_run = int(os.environ.get("KPAIRS", str(NPAIR)))
            fts, als = {}, {}

            # ---- phase 1: feat -> wsum -> al/alx for every pair (only
            # needs the small feat gathers, so it all runs early) ----
            def microb(src2, k):
                """[q, k] f32 (f16-pairs) -> [q, k, 64, 2] f16 view that
                repeats each pair 64x via a stride-0 middle dim; innermost
                stays stride-1 so DVE keeps the 2x perf mode."""
                return src2.bitcast(F16).rearrange(
                    "q (k two) -> q k two", two=2).unsqueeze(2).to_broadcast(
                    [128, k, 64, 2])

            for pair in range(npair_run):
                # weighted feat corners: one batched in-place TT per
                # (plane, 8-pair half); weights straight from the packed
                # f32 table via a stride-0 repeat view
                h = pair // 8
                if pair % 8 == 0:
                    for p in range(3):
                        v = gfeat[(p, h)][:].rearrange(
                            "q pr (cor c2 two) -> q (pr cor) c2 two",
                            cor=4, two=2)
                        nc.vector.tensor_mul(
                            v, v,
                            microb(gwf_t[:, p * 64 + h * 32:
                                         p * 64 + (h + 1) * 32], 32))
                gfr = [gfeat[(p, h)][:, pair % 8, :] for p in range(3)]
                # feat reduce: 3 slab accumulates (n=512) on PE, then a
                # 4->1 corner fold on DVE after the f16 evacuation
                psF4 = psmtpool.tile([128, 512], F32, tag="psF4")
                for i in range(3):
                    nc.tensor.matmul(psF4[:], ident_t[:], gfr[i],
                                     start=(i == 0), stop=(i == 2))
                F4 = mtpool.tile([128, 4, 128], F16, tag="F4")
                nc.scalar.copy(F4[:], psF4[:].rearrange(
                    "q (cor c) -> q cor c", cor=4))
                nc.vector.tensor_add(F4[:, 0:2, :], F4[:, 0:2, :],
                                     F4[:, 2:4, :])
                nc.vector.tensor_add(F4[:, 0, :], F4[:, 0, :], F4[:, 1, :])
                Fsb = F4[:, 0, :]

                # F^T for wsum matmul + final residual
                psFT = pmiscpool.tile([128, 128], F16, tag="psm")
                nc.tensor.matmul(psFT[:], Fsb, ident_t[:],
                                 is_transpose=True, start=True, stop=True)
                FTsb = ftpool.tile([128, 128], F16, tag="FTsb")
                nc.scalar.copy(FTsb[:], psFT[:])

                # wsum[s, a] = F @ W_wf  (lhsT = F^T)
                psW = psopool.tile([128, 128], F32, tag="psO")
                nc.tensor.matmul(psW[:, 0:NCP], FTsb[:], wwf_t[:],
                                 start=True, stop=True)
                WTsb = wtpool.tile([128, NCP], F16, tag="WTsb")
                nc.scalar.copy(WTsb[:], psW[:, 0:NCP])

                # al2[s, k] f32 = f16-pair (w4[s,k]*wsum[s,a(k)]) x2,
                # written directly by three per-plane 4D muls (DVE, tiny)
                al2 = alpool.tile([128, 96], F32, tag="al2")
                in1 = WTsb[:].unsqueeze(2).unsqueeze(3).to_broadcast(
                    [128, 8, 4, 2])
                for p in range(3):
                    out3 = al2[:, 32 * p:32 * (p + 1)].bitcast(F16).rearrange(
                        "q (a cor two) -> q a cor two", a=8, cor=4)
                    in0 = gwm_t[:, pair * 96 + 32 * p:pair * 96 + 32 * (p + 1)
                                ].rearrange("q (a cor) -> q a cor", a=8
                                            ).unsqueeze(3).to_broadcast(
                        [128, 8, 4, 2])
                    nc.vector.tensor_mul(out3, in0, in1)
                fts[pair], als[pair] = FTsb, al2

            # ---- phase 2: mix multiply + reduce + projection (short
            # tail behind each pair's mix gather) ----
            for pair in range(npair_run):
                FTsb = fts[pair]
                al2 = als[pair]
                # all planes in one tile: two half-size TT @2x in-place
                # muls, weights via stride-0 repeat view of packed pairs
                gmt = gmix[pair]
                ym4 = gmt[:].rearrange(
                    "q a (cor c2 two) -> q (a cor) c2 two", cor=4, two=2)
                nc.vector.tensor_mul(ym4[:, 0:48, :, :], ym4[:, 0:48, :, :],
                                     microb(al2[:, 0:48], 48))
                nc.vector.tensor_mul(ym4[:, 48:96, :, :], ym4[:, 48:96, :, :],
                                     microb(al2[:, 48:96], 48))
                # PE slab-accumulate (n=512): one matmul per anchor row,
                # summing over (plane, anchor) and keeping (cor, c)
                psM4 = psmtpool.tile([128, 512], F32, tag="psM4")
                for j in range(24):
                    nc.tensor.matmul(
                        psM4[:], ident_t[:], gmt[:, j, :],
                        start=(j == 0), stop=(j == 23))
                M4 = mtpool.tile([128, 4, 128], F16, tag="M4")
                nc.scalar.copy(M4[:], psM4[:].rearrange(
                    "q (cor c) -> q cor c", cor=4))
                nc.vector.tensor_add(M4[:, 0:2, :], M4[:, 0:2, :],
                                     M4[:, 2:4, :])
                nc.vector.tensor_add(M4[:, 0, :], M4[:, 0, :], M4[:, 1, :])
                psMT = pmiscpool.tile([128, 128], F16, tag="psm")
                nc.tensor.matmul(psMT[:], M4[:, 0, :], ident_t[:],
                                 is_transpose=True, start=True, stop=True)
                MTsb = mtpool.tile([128, 128], F16, tag="MTsb")
                nc.scalar.copy(MTsb[:], psMT[:])

                # out = M @ W_vo + F
                psO = psopool.tile([128, 128], F32, tag="psO")
                nc.tensor.matmul(psO[:], MTsb[:], wvo_t[:],
                                 start=True, stop=False)
                nc.tensor.matmul(psO[:], FTsb[:], ident_t[:],
                                 start=False, stop=True)
                Osb = opool.tile([128, 128], F32, tag="Osb")
                nc.scalar.copy(Osb[:], psO[:])
                nc.sync.dma_start(out_d[pair * 128:(pair + 1) * 128, :], Osb[:])
    nc.compile()
    return nc


def kernel(**inputs):
    core_inputs = _host_prep(inputs)
    if "nc" not in _CACHE:
        _CACHE["nc"] = _build()
    nc = _CACHE["nc"]
    trace = (os.environ.get("BASS_TRACE_KERNEL", "") not in ("", "0")
             and _install_ntff_hook())
    res = bass_utils.run_bass_kernel_spmd(
        nc, core_inputs, list(range(BS)), trace=trace)
    _CACHE["last_results"] = res
    outs = [np.asarray(res.results[i]["out"], dtype=np.float32)
            for i in range(BS)]
    return np.stack(outs, axis=0)
